# revision 13
# baseline (speedup 1.0000x reference)
"""CenterLoss (segment_reduce) Trainium2 kernel — fp8 single-load design.

Data-parallel over N across 8 cores; all feature traffic in fp8 e4m3
(rel-err budget 2e-2 is huge; measured error is ~1e-4 level).

Host prep (layout/dtype only):
  nat : [U*128, 544] fp8 lines.  Line (u, p) = [f(row(u,p,0)) | oh | pad |
        f(row(u,p,1)) | oh | pad] with row(u,p,t) = u*256 + t*128 + p and
        272B per t-half (256 f + 8 one-hot + 8 pad; 272 % 16 == 0 for the
        DoubleRow weight AP).  544B contiguous DMA descriptors (full rate).
  fta/ftb : [128, R] fp8 transposed feature halves (d major) for pass 2.

Device (per rep):
  Pass 1: per u-slot one DoubleRow matmul oh.T @ f (256 rows contracted per
    instruction) accumulating class sums [8, 256] in PSUM; per group one
    ones.T @ oh matmul for counts.
  f2 (row norms): chunks q in [0,NA) of each 64-chunk block squared on the
    Activation engine, [NA,NA+NP) on gpsimd, each reduced over d by a
    sq.T @ ones matmul on the PE (weights-stationary, FWL); remaining
    chunks via fused DVE tensor_tensor_reduce on the natural-layout tiles.
  AllReduce [8, 257] sums+counts; centers, W = -2*S*centers.T in fp8
    (S=256 scales centers into fp8 range), c2' = S*c2 + 240*empty.
  Pass 2: per 128-row chunk two FWL matmuls fT.T @ W into PSUM banks of 64
    chunks; c2' pre-added per bank via a rank-1 ones x c2rep matmul; DVE
    min-reduce over classes; d2 = mn/S + f2 (one scalar_tensor_tensor);
    one Sqrt activation with accum -> per-partition totals; ones matmul ->
    scalar.
"""

import numpy as np

from concourse import bass, bacc, mybir, tile
from concourse import bass_utils

F32 = mybir.dt.float32
BF16 = mybir.dt.bfloat16
FP8 = mybir.dt.float8e4
OP = mybir.AluOpType
AFT = mybir.ActivationFunctionType
PM = mybir.MatmulPerfMode

N_TOTAL = 524288
D = 256
C = 8
NCORES = 8
P = 128

LINE = 272            # per-t bytes: 256 f + 8 oh + 8 pad
GRP = 16              # u-slots per stage group
CPB = 64              # chunks per PSUM bank / block
S = 256.0             # fp8 scale for W / c2
BIGC = 240.0          # empty-class c2 mask (fp8 max finite)

NA = 24               # chunks per block squared on Activation
NP = 16               # chunks per block squared on gpsimd (Pool)
# remaining CPB-NA-NP chunks -> DVE tensor_tensor_reduce on natural tiles

USE_DR = True         # DoubleRow for pass-1 sums

import os
DBG = set(os.environ.get("KDBG", "").split(","))
if "nodr" in DBG:
    USE_DR = False


def build_nc(R: int, reps: int = 1):
    assert R % 256 == 0
    U = R // 256          # u-slots (256 rows each)
    NCHUNK = R // P       # 128-row chunks
    NBLK = NCHUNK // CPB
    NGRP = U // GRP
    nap = NA + NP

    nc = bacc.Bacc(
        "TRN2", target_bir_lowering=False, debug=False, num_devices=NCORES
    )
    nat_t = nc.dram_tensor("nat", [U * P, 2 * LINE], FP8, kind="ExternalInput")
    fta_t = nc.dram_tensor("fta", [P, R], FP8, kind="ExternalInput")
    ftb_t = nc.dram_tensor("ftb", [P, R], FP8, kind="ExternalInput")
    out_t = nc.dram_tensor("partial", [reps, 1], F32, kind="ExternalOutput")
    dbg_t = None
    if "dbg" in DBG:
        dbg_t = nc.dram_tensor("dbg", [reps * (C + P), D + 1], F32,
                               kind="ExternalOutput")

    with tile.TileContext(nc) as tc:
        with (
            tc.tile_pool(name="const", bufs=1) as constp,
            tc.tile_pool(name="stage", bufs=2) as stagep,
            tc.tile_pool(name="ft", bufs=NBLK) as ftp,
            tc.tile_pool(name="sq", bufs=2) as sqp,
            tc.tile_pool(name="ttr", bufs=2) as ttrp,
            tc.tile_pool(name="big", bufs=2) as bigp,
            tc.tile_pool(name="small", bufs=1) as smallp,
            tc.tile_pool(name="ps_acc", bufs=1, space="PSUM") as ps_accp,
            tc.tile_pool(name="ps_cnt", bufs=1, space="PSUM") as ps_cntp,
            tc.tile_pool(name="ps_f2", bufs=2, space="PSUM") as ps_f2p,
            tc.tile_pool(name="ps_fc", bufs=2, space="PSUM") as ps_fcp,
            tc.tile_pool(name="ps_small", bufs=2, space="PSUM") as ps_smallp,
            tc.tile_pool(name="dram", bufs=1, space="DRAM") as dramp,
        ):
            # ---------------- constants ----------------
            ones_c8 = constp.tile([P, 1], FP8)
            nc.vector.memset(ones_c8[:], 1.0)
            ones_r8 = constp.tile([1, P], FP8)
            nc.vector.memset(ones_r8[:], 1.0)
            ones_c32 = constp.tile([P, 1], F32)
            nc.vector.memset(ones_c32[:], 1.0)
            ones_c16 = constp.tile([P, 1], BF16)
            nc.vector.memset(ones_c16[:], 1.0)

            cls8 = constp.tile([C, C], F32)
            for c in range(C):
                nc.vector.memset(cls8[:, c : c + 1], float(c))
            pidx_i = constp.tile([C, 1], mybir.dt.int32)
            nc.gpsimd.iota(pidx_i[:], pattern=[[0, 1]], base=0,
                           channel_multiplier=1)
            pidx = constp.tile([C, 1], F32)
            nc.vector.tensor_copy(pidx[:], pidx_i[:])
            ident8 = constp.tile([C, C], F32)
            nc.vector.tensor_scalar(
                ident8[:], cls8[:], pidx[:], None, op0=OP.is_equal
            )

            for rep in range(reps):
                f2_all = bigp.tile([P, NCHUNK], F32, tag="f2all")
                mn_all = bigp.tile([P, NCHUNK], F32, tag="mnall")
                ps_sums = ps_accp.tile([C, D], F32, tag="ps_sums")
                ps_cnt = ps_cntp.tile([1, GRP * 2 * C], F32, tag="ps_cnt")

                fts = {}
                sqs = {}

                # ---- pass 1 + squares, interleaved group/block schedule ----
                def do_group(g):
                    st = stagep.tile([P, GRP, 2, LINE], FP8, tag="stage")
                    gsl = slice(g * GRP * P, (g + 1) * GRP * P)
                    nc.sync.dma_start(
                        st[:].rearrange("p u t l -> p u (t l)"),
                        nat_t.ap()[gsl, :].rearrange("(u p) b -> p u b", p=P),
                    )
                    for u in range(GRP):
                        ug = g * GRP + u
                        if USE_DR:
                            nc.tensor.matmul(
                                ps_sums[:],
                                st[:, u, :, 256:264],
                                st[:, u, :, 0:256],
                                start=(ug == 0), stop=(ug == U - 1),
                                perf_mode=PM.DoubleRow,
                            )
                        else:
                            for t in range(2):
                                nc.tensor.matmul(
                                    ps_sums[:],
                                    st[:, u, t, 256:264],
                                    st[:, u, t, 0:256],
                                    start=(ug == 0 and t == 0),
                                    stop=(ug == U - 1 and t == 1),
                                )
                    if "nocnt" not in DBG:
                        nc.tensor.matmul(
                            ps_cnt[:],
                            ones_c8[:],
                            st[:, :, :, 256:264],
                            start=(g == 0), stop=(g == NGRP - 1),
                        )
                    elif g == 0:
                        nc.vector.memset(ps_cnt[:], float(R) / (GRP * 2 * C))
                    # DVE share of f2 via fused square+row-reduce
                    for u in range(GRP):
                        ug = g * GRP + u
                        for t in range(2):
                            ch = 2 * ug + t
                            if ch % CPB >= nap:
                                if "nottr" in DBG:
                                    nc.vector.memset(
                                        f2_all[:, ch : ch + 1], 256.0
                                    )
                                    continue
                                # Fused DVE accum ops (tensor_tensor_
                                # reduce, scalar_tensor_tensor+accum)
                                # crash this trn2 runtime with fp8 ins;
                                # use plain square then reduce.
                                ts_ = ttrp.tile([P, D], BF16, tag="ttr")
                                nc.vector.tensor_tensor(
                                    ts_[:],
                                    st[:, u, t, 0:256],
                                    st[:, u, t, 0:256],
                                    op=OP.mult,
                                )
                                nc.vector.tensor_reduce(
                                    f2_all[:, ch : ch + 1], ts_[:],
                                    axis=mybir.AxisListType.X, op=OP.add,
                                )

                def do_block_load_sq(b):
                    ftA = ftp.tile([P, CPB * P], FP8, tag="ftA")
                    ftB = ftp.tile([P, CPB * P], FP8, tag="ftB")
                    bsl = slice(b * CPB * P, (b + 1) * CPB * P)
                    nc.sync.dma_start(ftA[:], fta_t.ap()[:, bsl])
                    nc.sync.dma_start(ftB[:], ftb_t.ap()[:, bsl])
                    fts[b] = (ftA, ftB)
                    sqA = sqp.tile([P, nap * P], BF16, tag="sqA")
                    sqB = sqp.tile([P, nap * P], BF16, tag="sqB")
                    for ft_, sq_ in ((ftA, sqA), (ftB, sqB)):
                        if "nopool" in DBG:
                            nc.scalar.activation(
                                sq_[:, 0 : nap * P], ft_[:, 0 : nap * P],
                                AFT.Square,
                            )
                            continue
                        nc.scalar.activation(
                            sq_[:, 0 : NA * P], ft_[:, 0 : NA * P], AFT.Square
                        )
                        nc.gpsimd.tensor_tensor(
                            sq_[:, NA * P : nap * P],
                            ft_[:, NA * P : nap * P],
                            ft_[:, NA * P : nap * P],
                            op=OP.mult,
                        )
                    sqs[b] = (sqA, sqB)

                def do_block_f2(b):
                    sqA, sqB = sqs[b]
                    ps_f2 = ps_f2p.tile([P, nap], F32, tag="ps_f2")
                    for q in range(nap):
                        o = ps_f2[:, q : q + 1]
                        nc.tensor.matmul(
                            o, sqA[:, q * P : (q + 1) * P], ones_c16[:],
                            start=True, stop=False,
                        )
                        nc.tensor.matmul(
                            o, sqB[:, q * P : (q + 1) * P], ones_c16[:],
                            start=False, stop=True,
                        )
                    nc.vector.tensor_copy(
                        f2_all[:, b * CPB : b * CPB + nap], ps_f2[:]
                    )

                # schedule: 2 groups then 1 block; late blocks' f2 reduces
                # are emitted after the collective is issued to fill the
                # PE bubble while the allreduce is in flight
                F2_INLINE = 5
                for b in range(NBLK):
                    do_group(2 * b)
                    do_group(2 * b + 1)
                    do_block_load_sq(b)
                    if b < F2_INLINE:
                        do_block_f2(b)

                # ---------------- allreduce ----------------
                payload = smallp.tile([C, D + 1], F32, tag="payload")
                nc.vector.tensor_copy(payload[:, 0:D], ps_sums[:])
                cnt_row = smallp.tile([1, C], F32, tag="cntrow")
                nc.vector.tensor_reduce(
                    cnt_row[:],
                    ps_cnt[:].rearrange("p (a c) -> p c a", c=C),
                    axis=mybir.AxisListType.X, op=OP.add,
                )
                ps_ct = ps_smallp.tile([C, 1], F32, tag="ps_small")
                nc.tensor.transpose(ps_ct[:], cnt_row[:], ident8[0:1, 0:1])
                nc.vector.tensor_copy(payload[:, D : D + 1], ps_ct[:])

                gsums = smallp.tile([C, D + 1], F32, tag="gsums")
                if "nocc" in DBG:
                    nc.vector.tensor_scalar_mul(gsums[:], payload[:],
                                                float(NCORES))
                else:
                    cc_in = dramp.tile([C, D + 1], F32, name=f"cci{rep}")
                    cc_out = dramp.tile([C, D + 1], F32, name=f"cco{rep}")
                    nc.gpsimd.dma_start(cc_in[:], payload[:])
                    nc.gpsimd.collective_compute(
                        "AllReduce", OP.add,
                        replica_groups=[list(range(NCORES))],
                        ins=[cc_in.opt()], outs=[cc_out.opt()],
                    )
                    nc.gpsimd.dma_start(gsums[:], cc_out[:])

                for b in range(F2_INLINE, NBLK):
                    do_block_f2(b)

                if dbg_t is not None:
                    base = rep * (C + P)
                    nc.sync.dma_start(
                        dbg_t.ap()[base : base + C, :], gsums[:]
                    )

                # ---------------- centers / weights ----------------
                counts = gsums[:, D : D + 1]
                cnt1 = smallp.tile([C, 1], F32, tag="cnt1")
                nc.vector.tensor_scalar_max(cnt1[:], counts, 1.0)
                recip = smallp.tile([C, 1], F32, tag="recip")
                nc.vector.reciprocal(recip[:], cnt1[:])
                centers = smallp.tile([C, D], F32, tag="centers")
                nc.vector.tensor_scalar(
                    centers[:], gsums[:, 0:D], recip[:], None, op0=OP.mult
                )
                ws = []
                for h in range(2):
                    ps_t = ps_smallp.tile([P, C], F32, tag="ps_small")
                    nc.tensor.transpose(
                        ps_t[:], centers[:, h * P : (h + 1) * P], ident8[:]
                    )
                    w = smallp.tile([P, C], FP8, tag=f"w{h}")
                    nc.vector.tensor_scalar_mul(w[:], ps_t[:], -2.0 * S)
                    ws.append(w)
                csq = smallp.tile([C, D], F32, tag="csq")
                nc.vector.tensor_tensor(csq[:], centers[:], centers[:],
                                        op=OP.mult)
                c2 = smallp.tile([C, 1], F32, tag="c2")
                nc.vector.tensor_reduce(
                    c2[:], csq[:], axis=mybir.AxisListType.X, op=OP.add
                )
                emptyb = smallp.tile([C, 1], F32, tag="emptyb")
                nc.vector.tensor_scalar(
                    emptyb[:], counts, 0.5, BIGC, op0=OP.is_lt, op1=OP.mult
                )
                c2s = smallp.tile([C, 1], F32, tag="c2s")
                nc.vector.scalar_tensor_tensor(
                    c2s[:], c2[:], S, emptyb[:], op0=OP.mult, op1=OP.add
                )
                ps_cr = ps_smallp.tile([1, C], F32, tag="ps_small")
                nc.tensor.transpose(ps_cr[:], c2s[:], ident8[:])
                c2r8 = smallp.tile([1, C], FP8, tag="c2r8")
                nc.vector.tensor_copy(c2r8[:], ps_cr[:])
                c2rep = smallp.tile([1, CPB * C], FP8, tag="c2rep")
                nc.vector.tensor_copy(c2rep[:, 0:C], c2r8[:])
                w_ = C
                while w_ < CPB * C:
                    nc.vector.tensor_copy(c2rep[:, w_ : 2 * w_],
                                          c2rep[:, 0:w_])
                    w_ *= 2

                # ---------------- pass 2 ----------------
                for b in range(NBLK):
                    ftA, ftB = fts[b]
                    ps_fc = ps_fcp.tile([P, CPB * C], F32, tag="ps_fc")
                    r1 = "nor1" not in DBG
                    if r1:
                        nc.tensor.matmul(
                            ps_fc[:], ones_r8[:], c2rep[:],
                            start=True, stop=False,
                        )
                    for q in range(CPB):
                        o = ps_fc[:, q * C : (q + 1) * C]
                        nc.tensor.matmul(
                            o, ftA[:, q * P : (q + 1) * P], ws[0][:],
                            start=not r1, stop=False,
                        )
                        nc.tensor.matmul(
                            o, ftB[:, q * P : (q + 1) * P], ws[1][:],
                            start=False, stop=True,
                        )
                    nc.vector.tensor_reduce(
                        mn_all[:, b * CPB : (b + 1) * CPB],
                        ps_fc[:].rearrange("p (t c) -> p t c", c=C),
                        axis=mybir.AxisListType.X, op=OP.min,
                    )

                # ---------------- d2, sqrt, total ----------------
                d2 = bigp.tile([P, NCHUNK], F32, tag="d2")
                nc.vector.scalar_tensor_tensor(
                    d2[:], mn_all[:], 1.0 / S, f2_all[:],
                    op0=OP.mult, op1=OP.add,
                )
                if dbg_t is not None:
                    base = rep * (C + P)
                    nc.sync.dma_start(
                        dbg_t.ap()[base + C : base + C + P, 0 : D + 1],
                        d2[:, 0 : D + 1],
                    )
                dsc = bigp.tile([P, NCHUNK], BF16, tag="dsc")
                tot = smallp.tile([P, 1], F32, tag="tot")
                nc.scalar.activation(
                    dsc[:], d2[:], AFT.Sqrt, accum_out=tot[:]
                )
                ps_tot = ps_smallp.tile([1, 1], F32, tag="ps_small")
                nc.tensor.matmul(
                    ps_tot[:], tot[:], ones_c32[:], start=True, stop=True
                )
                res = smallp.tile([1, 1], F32, tag="res")
                nc.vector.tensor_copy(res[:], ps_tot[:])
                nc.sync.dma_start(out_t.ap()[rep : rep + 1, :], res[:])

    nc.compile()
    return nc


_CACHE = {}


def _get_nc(R: int):
    if R not in _CACHE:
        _CACHE[R] = build_nc(R)
    return _CACHE[R]


def make_in_maps(features: np.ndarray, targets: np.ndarray,
                 ncores: int = NCORES):
    fp8np = mybir.dt.np(FP8)
    n = features.shape[0]
    r = n // ncores
    u = r // 256
    f8 = np.asarray(features, dtype=np.float32).astype(fp8np)
    tg = np.asarray(targets).astype(np.int64)
    in_maps = []
    for k in range(ncores):
        sl = slice(k * r, (k + 1) * r)
        f8c = f8[sl]
        tgc = tg[sl]
        nat = np.zeros((u, P, 2, LINE), dtype=fp8np)
        f3 = f8c.reshape(u, 2, P, D)          # (u, t, p, d)
        nat[:, :, :, 0:D] = f3.transpose(0, 2, 1, 3)
        oh = (tgc.reshape(u, 2, P)[:, :, :, None]
              == np.arange(C)[None, None, None, :]).astype(fp8np)
        nat[:, :, :, D : D + C] = oh.transpose(0, 2, 1, 3)
        ftc = np.ascontiguousarray(f8c.T)      # [256, r]
        in_maps.append({
            "nat": np.ascontiguousarray(nat.reshape(u * P, 2 * LINE)),
            "fta": np.ascontiguousarray(ftc[0:P]),
            "ftb": np.ascontiguousarray(ftc[P:D]),
        })
    return in_maps


def kernel(features, targets, **run_kwargs):
    features = np.asarray(features)
    targets = np.asarray(targets)
    n = features.shape[0]
    r = n // NCORES
    nc = _get_nc(r)
    in_maps = make_in_maps(features, targets)
    res = bass_utils.run_bass_kernel_spmd(
        nc, in_maps, core_ids=list(range(NCORES)), **run_kwargs
    )
    total = np.float64(0.0)
    for k in range(NCORES):
        total += np.float64(res.results[k]["partial"][0, 0])
    out = np.float32(total / n)
    if run_kwargs:
        return out, res
    return out


if __name__ == "__main__":
    nc = build_nc(65536)
    print("built OK")
